# revision 18
# baseline (speedup 1.0000x reference)
"""Trainium2 Bass kernel for qk-layernorm attention (dense transformer block).

Sharding: 8 cores = 2 batches x 4 head-groups (4 heads each).  Each core
computes qkv projection (its heads only), qk-layernorm, attention, and a
partial output projection for its head slice; the host sums the 4 partials
per batch and adds b_proj.

v2 design (all matmul operands bf16 -> FWL weight loads, no fp32_HIGH
serialization; PE kept dense to hold the HAM clock at 2.4 GHz):
 - x^T resident in SBUF (bf16), single pass: q/k projections via
   weight-stationary jobs, v projected directly into [m, d] layout using
   x^T tiles as the stationary operand (no PE transposes).
 - qk-layernorm stats via all-ones-block matmuls (broadcast rows in PSUM);
   1/sqrt via ACT Sqrt + DVE reciprocal_approx_fast; apply on GpSimd.
 - S^T per head pair with row-group concurrency (head A rows 0:64, head B
   rows 64:128 run concurrently in the PE array).
 - softmax without max subtraction (|logits| <= 8 since q,k are unit-var);
   exp on ACT (fp32 PSUM -> bf16), software-pipelined: S(mt+1) issued
   before attn@V(mt) so the PE never waits on the ACT exp.
 - denominator via ones-column in the attn@V stationary; broadcast via
   selector matmuls; one fast-reciprocal per head.
"""

import numpy as np

DIM = 1024
HEADS = 16
HD = 64
B = 2
N = 2048
EPS = 1e-6
N_CORES = 8
HEADS_PER_CORE = 4
PAIRS = 2          # head pairs per core
CC = 8             # contraction chunks of 128 over DIM
NT = N // 128      # 16 n/m tiles
NCH = N // 512     # 4 chunks of 512
SCALE = HD ** -0.5

_prog_cache = {}


def _build_program():
    import concourse.bass as bass
    import concourse.tile as tile
    from concourse import mybir, bacc

    F32 = mybir.dt.float32
    F16 = mybir.dt.float16
    BF16 = mybir.dt.bfloat16
    Act = mybir.ActivationFunctionType
    Alu = mybir.AluOpType

    nc = bacc.Bacc("TRN2", target_bir_lowering=False, debug=False,
                   num_devices=N_CORES)

    # ---- DRAM I/O (bf16 unless noted) ----
    xT_d = [nc.dram_tensor(f"xT{i}", [DIM, 512], BF16, kind="ExternalInput").ap()
            for i in range(NCH)]
    wqk_d = nc.dram_tensor("wqk", [DIM, 512], BF16, kind="ExternalInput").ap()
    wv_d = nc.dram_tensor("wv", [DIM, 256], BF16, kind="ExternalInput").ap()
    wp_d = nc.dram_tensor("wp", [256, DIM], BF16, kind="ExternalInput").ap()
    smu_q_d = nc.dram_tensor("smu_q", [128, 128], BF16, kind="ExternalInput").ap()
    ssq_q_d = nc.dram_tensor("ssq_q", [128, 128], BF16, kind="ExternalInput").ap()
    smu_k_d = nc.dram_tensor("smu_k", [128, 128], BF16, kind="ExternalInput").ap()
    ssq_k_d = nc.dram_tensor("ssq_k", [128, 128], BF16, kind="ExternalInput").ap()
    gq_d = nc.dram_tensor("gq", [128, 1], F32, kind="ExternalInput").ap()
    gk_d = nc.dram_tensor("gk", [128, 1], F32, kind="ExternalInput").ap()
    y_d = nc.dram_tensor("y", [N, DIM], BF16, kind="ExternalOutput").ap()

    with tile.TileContext(nc) as tc:
        with tc.tile_pool(name="wts", bufs=1) as wts, \
             tc.tile_pool(name="persist", bufs=1) as persist:
            # ---- persistent SBUF tensors ----
            wqkt = [wts.tile([128, 4 * 512], BF16, tag=f"wqk{i}",
                             name=f"wqk{i}") for i in range(2)]
            for i in range(2):
                nc.sync.dma_start(
                    wqkt[i][:].rearrange("p (cc o) -> p cc o", cc=4),
                    wqk_d[i * 512:(i + 1) * 512, :]
                    .rearrange("(cc p) o -> p cc o", p=128))
            wv = wts.tile([128, CC * 256], BF16)
            nc.sync.dma_start(wv[:].rearrange("p (cc o) -> p cc o", cc=CC),
                              wv_d.rearrange("(cc p) o -> p cc o", p=128))
            xT = [wts.tile([128, CC * 512], BF16, tag=f"xT{i}", name=f"xT{i}")
                  for i in range(NCH)]
            for i in range(NCH):
                nc.sync.dma_start(
                    xT[i][:].rearrange("p (cc n) -> p cc n", cc=CC),
                    xT_d[i].rearrange("(cc p) n -> p cc n", p=128))
            wp = wts.tile([128, 2 * DIM], BF16)
            nc.sync.dma_start(wp[:].rearrange("p (pc o) -> p pc o", pc=2),
                              wp_d.rearrange("(pc p) o -> p pc o", p=128))
            smu = [wts.tile([128, 128], BF16, tag=f"smu{t}", name=f"smu{t}") for t in range(2)]
            ssq = [wts.tile([128, 128], BF16, tag=f"ssq{t}", name=f"ssq{t}") for t in range(2)]
            nc.sync.dma_start(smu[0][:], smu_q_d[:])
            nc.sync.dma_start(ssq[0][:], ssq_q_d[:])
            nc.sync.dma_start(smu[1][:], smu_k_d[:])
            nc.sync.dma_start(ssq[1][:], ssq_k_d[:])
            gcol = [wts.tile([128, 1], F32, tag=f"g{t}", name=f"g{t}") for t in range(2)]
            nc.sync.dma_start(gcol[0][:], gq_d[:])
            nc.sync.dma_start(gcol[1][:], gk_d[:])
            epsb = wts.tile([128, 1], F32)
            nc.gpsimd.memset(epsb[:], EPS)

            # qk[0],qk[1]: q pair tiles; qk[2],qk[3]: k pair tiles  [d-pair, n]
            qk = [persist.tile([128, N], BF16, tag=f"qk{i}", name=f"qk{i}") for i in range(4)]
            # v_sb: [m, nt*(4 heads x [64 v | 1])]
            v_sb = persist.tile([128, NT * 260], BF16)
            nc.vector.memset(
                v_sb[:].rearrange("p (nt h c) -> p nt h c", nt=NT, h=4)[:, :, :, 64:65],
                1.0)
            onorm = [persist.tile([128, N], BF16, tag=f"on{p}", name=f"on{p}") for p in range(PAIRS)]

            # ================= Phase 1: qkv projection + qk layernorm ========
            with tc.tile_pool(name="p1tmp", bufs=2) as tmp, \
                 tc.tile_pool(name="ps1", bufs=1, space="PSUM") as ps1:
                for nch in range(NCH):
                    sl = slice(nch * 512, (nch + 1) * 512)
                    xt = xT[nch][:].rearrange("p (cc n) -> p cc n", cc=CC)
                    # q/k projection: 4 jobs (qp0, qp1, kp0, kp1)
                    accs = [ps1.tile([128, 512], F32, tag=f"acc{j}",
                                     name=f"acc{j}") for j in range(4)]
                    for cc in range(CC):
                        for j in range(4):
                            nc.tensor.matmul(
                                accs[j][:],
                                wqkt[cc // 4][:, (cc % 4) * 512 + j * 128:
                                              (cc % 4) * 512 + (j + 1) * 128],
                                xt[:, cc, :],
                                start=(cc == 0), stop=(cc == CC - 1))
                    # qk layernorm per (pair, t); stats via matmul
                    for t in (1, 0):        # k first: attention needs k for all n
                        for p in range(PAIRS):
                            j = 2 * t + p
                            src = qk[j]
                            nc.scalar.copy(src[:, sl], accs[j][:])
                            sqc = tmp.tile([128, 512], BF16, tag="sqc",
                                           name="sqc")
                            nc.gpsimd.tensor_mul(sqc[:], src[:, sl], src[:, sl])
                            pmu = ps1.tile([128, 512], F32, tag="mu", name="pmu")
                            psq = ps1.tile([128, 512], F32, tag="sq", name="psq")
                            nc.tensor.matmul(pmu[:], smu[t][:], src[:, sl],
                                             start=True, stop=True)
                            nc.tensor.matmul(psq[:], ssq[t][:], sqc[:],
                                             start=True, stop=True)
                            t1 = tmp.tile([128, 512], F32, tag="t1", name="t1")
                            nc.vector.tensor_sub(t1[:], src[:, sl], pmu[:])
                            sqs = tmp.tile([128, 512], F32, tag="sqs", name="sqs")
                            nc.scalar.activation(sqs[:], pmu[:], Act.Square)
                            var = tmp.tile([128, 512], F32, tag="var", name="var")
                            nc.vector.tensor_sub(var[:], psq[:], sqs[:])
                            # sig = sqrt(var/g^2 + eps) = sqrt(var_raw + eps)
                            sig = tmp.tile([128, 512], F32, tag="sig", name="sig")
                            nc.scalar.activation(sig[:], var[:], Act.Sqrt,
                                                 scale=gcol[t][:], bias=epsb[:])
                            rs = tmp.tile([128, 512], F32, tag="rs", name="rs")
                            nc.vector.reciprocal_approx_fast(rs[:], sig[:])
                            nc.gpsimd.tensor_mul(src[:, sl], t1[:], rs[:])

                    # v direct: stationary = x^T m-tile, moving = wv
                    for ml in range(4):
                        mt = nch * 4 + ml
                        accv = ps1.tile([128, 256], F32, tag="av", bufs=2,
                                        name="accv")
                        for cc in range(CC):
                            nc.tensor.matmul(
                                accv[:],
                                xt[:, cc, ml * 128:(ml + 1) * 128],
                                wv[:, cc * 256:(cc + 1) * 256],
                                start=(cc == 0), stop=(cc == CC - 1))
                        for p in range(PAIRS):
                            nc.vector.tensor_copy(
                                v_sb[:, mt * 260 + p * 130: mt * 260 + p * 130 + 130]
                                .rearrange("q (h c) -> q h c", h=2)[:, :, 0:64],
                                accv[:, p * 128:(p + 1) * 128]
                                .rearrange("q (h c) -> q h c", h=2))
            # ================= Phase 3: attention =================
            PYTAGS = ["s0", "s1", "o0", "o1"]

            def proj_tiles(p4, ps4, nt):
                py = ps4.tile([128, 1024], F32, tag=PYTAGS[nt % 4],
                              bufs=1, name="py")
                for p in range(PAIRS):
                    for oc in range(2):
                        nc.tensor.matmul(
                            py[:, oc * 512:(oc + 1) * 512],
                            onorm[p][:, nt * 128:(nt + 1) * 128],
                            wp[:, p * 1024 + oc * 512:p * 1024 + (oc + 1) * 512],
                            start=(p == 0), stop=(p == PAIRS - 1))
                yt = p4.tile([128, 1024], BF16, tag="yt", bufs=6, name="yt")
                for oc in range(2):
                    half = slice(oc * 512, (oc + 1) * 512)
                    if (2 * nt + oc) % 2 == 0:
                        nc.scalar.copy(yt[:, half], py[:, half])
                    else:
                        nc.vector.tensor_copy(yt[:, half], py[:, half])
                nc.sync.dma_start(y_d[nt * 128:(nt + 1) * 128, :], yt[:])

            with tc.tile_pool(name="p3", bufs=2) as p3, \
                 tc.tile_pool(name="ps3", bufs=1, space="PSUM") as ps3:
                for nh in range(2):            # halves of n (1024 each)
                    for p in range(PAIRS):
                        qt, kt = qk[p], qk[2 + p]
                        poh = [ps3.tile([128, 1024], F32, tag=f"o{h}",
                                        name=f"po{h}") for h in range(2)]
                        eSs = [None, None]

                        def s_exp(mt):
                            # S^T matmuls: head A rows 0:64, head B 64:128
                            # run concurrently (distinct PE row groups)
                            psS = [ps3.tile([128, 1024], F32, tag=f"s{h}",
                                            bufs=1, name=f"psS{h}")
                                   for h in range(2)]
                            for nq in range(2):
                                for h in range(2):
                                    hs = slice(h * 64, (h + 1) * 64)
                                    nsl = slice(nh * 1024 + nq * 512,
                                                nh * 1024 + (nq + 1) * 512)
                                    nc.tensor.matmul(
                                        psS[h][:, nq * 512:(nq + 1) * 512],
                                        kt[hs, mt * 128:(mt + 1) * 128],
                                        qt[hs, nsl], start=True, stop=True)
                            for h in range(2):
                                eS = p3.tile([128, 1024], BF16, tag=f"eS{h}",
                                             bufs=3, name=f"eS{h}")
                                nc.scalar.activation(eS[:], psS[h][:], Act.Exp,
                                                     scale=float(SCALE))
                                eSs[h] = eS

                        def attn_v(mt, eS_prev):
                            first, last = (mt == 0), (mt == NT - 1)
                            for h in range(2):
                                vsl = v_sb[:, mt * 260 + (p * 2 + h) * 65:
                                           mt * 260 + (p * 2 + h) * 65 + 65]
                                for nq in range(2):
                                    nc.tensor.matmul(
                                        poh[h][0:65, nq * 512:(nq + 1) * 512],
                                        vsl,
                                        eS_prev[h][:, nq * 512:(nq + 1) * 512],
                                        start=first, stop=last)

                        # software pipeline: S(mt+1) issued before attnV(mt)
                        s_exp(0)
                        prev = list(eSs)
                        for mt in range(1, NT):
                            s_exp(mt)
                            attn_v(mt - 1, prev)
                            prev = list(eSs)
                        attn_v(NT - 1, prev)

                        # denominators -> reciprocal on compact rows ->
                        # partition-broadcast (GpSimd) -> normalize.  No PSUM
                        # or PE involvement, so the next pair's S matmuls can
                        # start as soon as the psS banks free up.
                        # copy poh out of PSUM immediately (frees the o
                        # banks for the next pair), then normalize off SBUF
                        osb = [p3.tile([128, 1024], F32, tag=f"osb{h}",
                                       name=f"osb{h}") for h in range(2)]
                        for h in range(2):
                            nc.vector.tensor_copy(osb[h][0:65, :],
                                                  poh[h][0:65, :])
                        rd = []
                        for h in range(2):
                            dn = p3.tile([128, 1024], F32, tag=f"dn{h}",
                                         name=f"dn{h}")
                            nc.sync.dma_start(dn[0:1, :], osb[h][64:65, :])
                            nc.vector.reciprocal_approx_fast(
                                dn[0:1, :], dn[0:1, :])
                            rdh = p3.tile([128, 1024], F32, tag=f"rd{h}",
                                          name=f"rd{h}")
                            nc.gpsimd.partition_broadcast(
                                rdh[0:64, :], dn[0:1, :], channels=64)
                            rd.append(rdh)
                        nc.vector.tensor_mul(
                            onorm[p][0:64, nh * 1024:(nh + 1) * 1024],
                            osb[0][0:64, :], rd[0][0:64, :])
                        tmpB = p3.tile([128, 1024], BF16, tag="tmpB")
                        nc.vector.tensor_mul(
                            tmpB[0:64, :], osb[1][0:64, :], rd[1][0:64, :])
                        nc.sync.dma_start(
                            onorm[p][64:128, nh * 1024:(nh + 1) * 1024],
                            tmpB[0:64, :])
                with tc.tile_pool(name="p4", bufs=1) as p4:
                    for nt in range(NT):
                        proj_tiles(p4, ps3, nt)

    nc.compile()
    return nc


def _prep_core_inputs(x, W_qkv, q_gamma, k_gamma, W_proj):
    """Host-side sharding + layout prep. Returns list of 8 in_maps."""
    import ml_dtypes
    bf16 = ml_dtypes.bfloat16
    f32 = np.float32
    blkdiag = np.kron(np.eye(2, dtype=f32), np.ones((64, 64), f32))
    in_maps = []
    for core in range(N_CORES):
        b, g = core // 4, core % 4
        heads = [4 * g + j for j in range(HEADS_PER_CORE)]
        qcols = np.concatenate(
            [(W_qkv[h * HD:(h + 1) * HD, :] * q_gamma[:, None]).T for h in heads],
            axis=1)
        kcols = np.concatenate(
            [(W_qkv[DIM + h * HD:DIM + (h + 1) * HD, :] * k_gamma[:, None]).T
             for h in heads], axis=1)
        wqk = np.ascontiguousarray(
            np.concatenate([qcols, kcols], axis=1)).astype(bf16)
        wv = np.ascontiguousarray(
            np.concatenate(
                [W_qkv[2 * DIM + h * HD:2 * DIM + (h + 1) * HD, :].T
                 for h in heads], axis=1)).astype(bf16)
        wp = np.ascontiguousarray(
            W_proj[:, heads[0] * HD:(heads[-1] + 1) * HD].T).astype(bf16)
        g2q = np.tile(q_gamma, 2).astype(f32)
        g2k = np.tile(k_gamma, 2).astype(f32)
        xTb = np.ascontiguousarray(x[b].T).astype(bf16)
        im = {
            "wqk": wqk, "wv": wv, "wp": wp,
            "smu_q": (blkdiag / 64.0).astype(bf16),
            "ssq_q": (blkdiag / 64.0).astype(bf16),
            "smu_k": (blkdiag / 64.0).astype(bf16),
            "ssq_k": (blkdiag / 64.0).astype(bf16),
            "gq": (1.0 / (g2q * g2q))[:, None].astype(f32),
            "gk": (1.0 / (g2k * g2k))[:, None].astype(f32),
        }
        for i in range(NCH):
            im[f"xT{i}"] = np.ascontiguousarray(xTb[:, i * 512:(i + 1) * 512])
        in_maps.append(im)
    return in_maps


def _numpy_fallback(x, W_qkv, q_gamma, q_beta, k_gamma, k_beta, W_proj, b_proj):
    def ln(t, gamma, beta):
        mu = t.mean(-1, keepdims=True)
        var = ((t - mu) ** 2).mean(-1, keepdims=True)
        return (t - mu) / np.sqrt(var + EPS) * gamma + beta
    Bs, Ns, C = x.shape
    qkv = np.einsum('bnc,oc->bno', x, W_qkv)
    qkv = qkv.reshape(Bs, Ns, 3, HEADS, HD).transpose(2, 0, 3, 1, 4)
    q, k, v = ln(qkv[0], q_gamma, q_beta), ln(qkv[1], k_gamma, k_beta), qkv[2]
    s = np.einsum('bhnd,bhmd->bhnm', q * SCALE, k)
    s = np.exp(s - s.max(-1, keepdims=True))
    p = s / s.sum(-1, keepdims=True)
    o = np.einsum('bhnm,bhmd->bhnd', p, v)
    o = o.transpose(0, 2, 1, 3).reshape(Bs, Ns, C)
    return (np.einsum('bnc,oc->bno', o, W_proj) + b_proj).astype(np.float32)


def kernel(x, W_qkv, q_gamma, q_beta, k_gamma, k_beta, W_proj, b_proj):
    x = np.asarray(x, np.float32)
    W_qkv = np.asarray(W_qkv, np.float32)
    q_gamma = np.asarray(q_gamma, np.float32)
    q_beta = np.asarray(q_beta, np.float32)
    k_gamma = np.asarray(k_gamma, np.float32)
    k_beta = np.asarray(k_beta, np.float32)
    W_proj = np.asarray(W_proj, np.float32)
    b_proj = np.asarray(b_proj, np.float32)

    if np.any(q_beta != 0) or np.any(k_beta != 0):
        # beta terms are not wired into the device kernel (reference always
        # uses beta = 0); fall back to exact host computation
        return _numpy_fallback(x, W_qkv, q_gamma, q_beta, k_gamma, k_beta,
                               W_proj, b_proj)

    from concourse import bass_utils

    if "prog" not in _prog_cache:
        _prog_cache["prog"] = _build_program()
    nc = _prog_cache["prog"]

    in_maps = _prep_core_inputs(x, W_qkv, q_gamma, k_gamma, W_proj)
    res = bass_utils.run_bass_kernel_spmd(nc, in_maps, list(range(N_CORES)))

    out = np.empty((B, N, DIM), np.float32)
    for b in range(B):
        acc = res.results[4 * b + 0]["y"].astype(np.float32).copy()
        for g in range(1, 4):
            acc += res.results[4 * b + g]["y"]
        out[b] = acc + b_proj
    return out


# revision 19
# speedup vs baseline: 1.1168x; 1.1168x over previous
"""Trainium2 Bass kernel for qk-layernorm attention (dense transformer block).

Sharding: 8 cores = 2 batches x 4 head-groups (4 heads each).  Each core
computes qkv projection (its heads only), qk-layernorm, attention, and a
partial output projection for its head slice; the host sums the 4 partials
per batch and adds b_proj.

v2 design (all matmul operands bf16 -> FWL weight loads, no fp32_HIGH
serialization; PE kept dense to hold the HAM clock at 2.4 GHz):
 - x^T resident in SBUF (bf16), single pass: q/k projections via
   weight-stationary jobs, v projected directly into [m, d] layout using
   x^T tiles as the stationary operand (no PE transposes).
 - qk-layernorm stats via all-ones-block matmuls (broadcast rows in PSUM);
   1/sqrt via ACT Sqrt + DVE reciprocal_approx_fast; apply on GpSimd.
 - S^T per head pair with row-group concurrency (head A rows 0:64, head B
   rows 64:128 run concurrently in the PE array).
 - softmax without max subtraction (|logits| <= 8 since q,k are unit-var);
   exp on ACT (fp32 PSUM -> bf16), software-pipelined: S(mt+1) issued
   before attn@V(mt) so the PE never waits on the ACT exp.
 - denominator via ones-column in the attn@V stationary; broadcast via
   selector matmuls; one fast-reciprocal per head.
"""

import numpy as np

DIM = 1024
HEADS = 16
HD = 64
B = 2
N = 2048
EPS = 1e-6
N_CORES = 8
HEADS_PER_CORE = 4
PAIRS = 2          # head pairs per core
CC = 8             # contraction chunks of 128 over DIM
NT = N // 128      # 16 n/m tiles
NCH = N // 512     # 4 chunks of 512
SCALE = HD ** -0.5

_prog_cache = {}


def _build_program():
    import concourse.bass as bass
    import concourse.tile as tile
    from concourse import mybir, bacc

    F32 = mybir.dt.float32
    F16 = mybir.dt.float16
    BF16 = mybir.dt.bfloat16
    Act = mybir.ActivationFunctionType
    Alu = mybir.AluOpType

    nc = bacc.Bacc("TRN2", target_bir_lowering=False, debug=False,
                   num_devices=N_CORES)

    # ---- DRAM I/O (bf16 unless noted) ----
    xT_d = [nc.dram_tensor(f"xT{i}", [DIM, 512], BF16, kind="ExternalInput").ap()
            for i in range(NCH)]
    wqk_d = nc.dram_tensor("wqk", [DIM, 512], BF16, kind="ExternalInput").ap()
    wv_d = nc.dram_tensor("wv", [DIM, 256], BF16, kind="ExternalInput").ap()
    wp_d = nc.dram_tensor("wp", [256, DIM], BF16, kind="ExternalInput").ap()
    smu_q_d = nc.dram_tensor("smu_q", [128, 128], BF16, kind="ExternalInput").ap()
    ssq_q_d = nc.dram_tensor("ssq_q", [128, 128], BF16, kind="ExternalInput").ap()
    smu_k_d = nc.dram_tensor("smu_k", [128, 128], BF16, kind="ExternalInput").ap()
    ssq_k_d = nc.dram_tensor("ssq_k", [128, 128], BF16, kind="ExternalInput").ap()
    gq_d = nc.dram_tensor("gq", [128, 1], F32, kind="ExternalInput").ap()
    gk_d = nc.dram_tensor("gk", [128, 1], F32, kind="ExternalInput").ap()
    y_d = nc.dram_tensor("y", [N, DIM], BF16, kind="ExternalOutput").ap()

    with tile.TileContext(nc) as tc:
        with tc.tile_pool(name="wts", bufs=1) as wts, \
             tc.tile_pool(name="persist", bufs=1) as persist:
            # ---- persistent SBUF tensors ----
            wqkt = [wts.tile([128, 4 * 512], BF16, tag=f"wqk{i}",
                             name=f"wqk{i}") for i in range(2)]
            for i in range(2):
                nc.sync.dma_start(
                    wqkt[i][:].rearrange("p (cc o) -> p cc o", cc=4),
                    wqk_d[i * 512:(i + 1) * 512, :]
                    .rearrange("(cc p) o -> p cc o", p=128))
            wv = wts.tile([128, CC * 256], BF16)
            nc.sync.dma_start(wv[:].rearrange("p (cc o) -> p cc o", cc=CC),
                              wv_d.rearrange("(cc p) o -> p cc o", p=128))
            xT = [wts.tile([128, CC * 512], BF16, tag=f"xT{i}", name=f"xT{i}")
                  for i in range(NCH)]
            for i in range(NCH):
                nc.sync.dma_start(
                    xT[i][:].rearrange("p (cc n) -> p cc n", cc=CC),
                    xT_d[i].rearrange("(cc p) n -> p cc n", p=128))
            wp = wts.tile([128, 2 * DIM], BF16)
            nc.sync.dma_start(wp[:].rearrange("p (pc o) -> p pc o", pc=2),
                              wp_d.rearrange("(pc p) o -> p pc o", p=128))
            smu = [wts.tile([128, 128], BF16, tag=f"smu{t}", name=f"smu{t}") for t in range(2)]
            ssq = [wts.tile([128, 128], BF16, tag=f"ssq{t}", name=f"ssq{t}") for t in range(2)]
            nc.sync.dma_start(smu[0][:], smu_q_d[:])
            nc.sync.dma_start(ssq[0][:], ssq_q_d[:])
            nc.sync.dma_start(smu[1][:], smu_k_d[:])
            nc.sync.dma_start(ssq[1][:], ssq_k_d[:])
            gcol = [wts.tile([128, 1], F32, tag=f"g{t}", name=f"g{t}") for t in range(2)]
            nc.sync.dma_start(gcol[0][:], gq_d[:])
            nc.sync.dma_start(gcol[1][:], gk_d[:])
            epsb = wts.tile([128, 1], F32)
            nc.gpsimd.memset(epsb[:], EPS)

            # qk[0],qk[1]: q pair tiles; qk[2],qk[3]: k pair tiles  [d-pair, n]
            qk = [persist.tile([128, N], BF16, tag=f"qk{i}", name=f"qk{i}") for i in range(4)]
            # v_sb: [m, nt*(4 heads x [64 v | 1])]
            v_sb = persist.tile([128, NT * 260], BF16)
            nc.vector.memset(
                v_sb[:].rearrange("p (nt h c) -> p nt h c", nt=NT, h=4)[:, :, :, 64:65],
                1.0)
            onorm = [persist.tile([128, N], BF16, tag=f"on{p}", name=f"on{p}") for p in range(PAIRS)]

            # ================= Phase 1: qkv projection + qk layernorm ========
            with tc.tile_pool(name="p1tmp", bufs=2) as tmp, \
                 tc.tile_pool(name="ps1", bufs=1, space="PSUM") as ps1:
                for nch in range(NCH):
                    sl = slice(nch * 512, (nch + 1) * 512)
                    xt = xT[nch][:].rearrange("p (cc n) -> p cc n", cc=CC)
                    # q/k projection: 4 jobs (qp0, qp1, kp0, kp1)
                    accs = [ps1.tile([128, 512], F32, tag=f"acc{j}",
                                     name=f"acc{j}") for j in range(4)]
                    for cc in range(CC):
                        for j in range(4):
                            nc.tensor.matmul(
                                accs[j][:],
                                wqkt[cc // 4][:, (cc % 4) * 512 + j * 128:
                                              (cc % 4) * 512 + (j + 1) * 128],
                                xt[:, cc, :],
                                start=(cc == 0), stop=(cc == CC - 1))
                    # v direct: stationary = x^T m-tile, moving = wv
                    for ml in range(4):
                        mt = nch * 4 + ml
                        accv = ps1.tile([128, 256], F32, tag="av", bufs=2,
                                        name="accv")
                        for cc in range(CC):
                            nc.tensor.matmul(
                                accv[:],
                                xt[:, cc, ml * 128:(ml + 1) * 128],
                                wv[:, cc * 256:(cc + 1) * 256],
                                start=(cc == 0), stop=(cc == CC - 1))
                        for p in range(PAIRS):
                            nc.vector.tensor_copy(
                                v_sb[:, mt * 260 + p * 130: mt * 260 + p * 130 + 130]
                                .rearrange("q (h c) -> q h c", h=2)[:, :, 0:64],
                                accv[:, p * 128:(p + 1) * 128]
                                .rearrange("q (h c) -> q h c", h=2))
                    # qk layernorm per (pair, t); stats via matmul
                    for t in (1, 0):        # k first: attention needs k for all n
                        for p in range(PAIRS):
                            j = 2 * t + p
                            src = qk[j]
                            nc.scalar.copy(src[:, sl], accs[j][:])
                            sqc = tmp.tile([128, 512], BF16, tag="sqc",
                                           name="sqc")
                            nc.gpsimd.tensor_mul(sqc[:], src[:, sl], src[:, sl])
                            pmu = ps1.tile([128, 512], F32, tag="mu", name="pmu")
                            psq = ps1.tile([128, 512], F32, tag="sq", name="psq")
                            nc.tensor.matmul(pmu[:], smu[t][:], src[:, sl],
                                             start=True, stop=True)
                            nc.tensor.matmul(psq[:], ssq[t][:], sqc[:],
                                             start=True, stop=True)
                            t1 = tmp.tile([128, 512], F32, tag="t1", name="t1")
                            nc.vector.tensor_sub(t1[:], src[:, sl], pmu[:])
                            sqs = tmp.tile([128, 512], F32, tag="sqs", name="sqs")
                            nc.scalar.activation(sqs[:], pmu[:], Act.Square)
                            var = tmp.tile([128, 512], F32, tag="var", name="var")
                            nc.vector.tensor_sub(var[:], psq[:], sqs[:])
                            # sig = sqrt(var/g^2 + eps) = sqrt(var_raw + eps)
                            sig = tmp.tile([128, 512], F32, tag="sig", name="sig")
                            nc.scalar.activation(sig[:], var[:], Act.Sqrt,
                                                 scale=gcol[t][:], bias=epsb[:])
                            rs = tmp.tile([128, 512], F32, tag="rs", name="rs")
                            nc.vector.reciprocal_approx_fast(rs[:], sig[:])
                            nc.gpsimd.tensor_mul(src[:, sl], t1[:], rs[:])

            # ================= Phase 3: attention =================
            PYTAGS = ["s0", "s1", "o0", "o1"]

            def proj_tiles(p4, ps4, nt):
                py = ps4.tile([128, 1024], F32, tag=PYTAGS[nt % 4],
                              bufs=1, name="py")
                for p in range(PAIRS):
                    for oc in range(2):
                        nc.tensor.matmul(
                            py[:, oc * 512:(oc + 1) * 512],
                            onorm[p][:, nt * 128:(nt + 1) * 128],
                            wp[:, p * 1024 + oc * 512:p * 1024 + (oc + 1) * 512],
                            start=(p == 0), stop=(p == PAIRS - 1))
                yt = p4.tile([128, 1024], BF16, tag="yt", bufs=6, name="yt")
                for oc in range(2):
                    half = slice(oc * 512, (oc + 1) * 512)
                    if (2 * nt + oc) % 2 == 0:
                        nc.scalar.copy(yt[:, half], py[:, half])
                    else:
                        nc.vector.tensor_copy(yt[:, half], py[:, half])
                nc.sync.dma_start(y_d[nt * 128:(nt + 1) * 128, :], yt[:])

            with tc.tile_pool(name="p3", bufs=2) as p3, \
                 tc.tile_pool(name="ps3", bufs=1, space="PSUM") as ps3:
                for nh in range(2):            # halves of n (1024 each)
                    for p in range(PAIRS):
                        qt, kt = qk[p], qk[2 + p]
                        poh = [ps3.tile([128, 1024], F32, tag=f"o{h}",
                                        name=f"po{h}") for h in range(2)]
                        eSs = [None, None]

                        def s_exp(mt):
                            # S^T matmuls: head A rows 0:64, head B 64:128
                            # run concurrently (distinct PE row groups)
                            psS = [ps3.tile([128, 1024], F32, tag=f"s{h}",
                                            bufs=1, name=f"psS{h}")
                                   for h in range(2)]
                            for nq in range(2):
                                for h in range(2):
                                    hs = slice(h * 64, (h + 1) * 64)
                                    nsl = slice(nh * 1024 + nq * 512,
                                                nh * 1024 + (nq + 1) * 512)
                                    nc.tensor.matmul(
                                        psS[h][:, nq * 512:(nq + 1) * 512],
                                        kt[hs, mt * 128:(mt + 1) * 128],
                                        qt[hs, nsl], start=True, stop=True)
                            for h in range(2):
                                eS = p3.tile([128, 1024], BF16, tag=f"eS{h}",
                                             bufs=3, name=f"eS{h}")
                                nc.scalar.activation(eS[:], psS[h][:], Act.Exp,
                                                     scale=float(SCALE))
                                eSs[h] = eS

                        def attn_v(mt, eS_prev):
                            first, last = (mt == 0), (mt == NT - 1)
                            for h in range(2):
                                vsl = v_sb[:, mt * 260 + (p * 2 + h) * 65:
                                           mt * 260 + (p * 2 + h) * 65 + 65]
                                for nq in range(2):
                                    nc.tensor.matmul(
                                        poh[h][0:65, nq * 512:(nq + 1) * 512],
                                        vsl,
                                        eS_prev[h][:, nq * 512:(nq + 1) * 512],
                                        start=first, stop=last)

                        # software pipeline: S(mt+1) issued before attnV(mt)
                        s_exp(0)
                        prev = list(eSs)
                        for mt in range(1, NT):
                            s_exp(mt)
                            attn_v(mt - 1, prev)
                            prev = list(eSs)
                        attn_v(NT - 1, prev)

                        # denominators -> reciprocal on compact rows ->
                        # partition-broadcast (GpSimd) -> normalize.  No PSUM
                        # or PE involvement, so the next pair's S matmuls can
                        # start as soon as the psS banks free up.
                        # copy poh out of PSUM immediately (frees the o
                        # banks for the next pair), then normalize off SBUF
                        osb = [p3.tile([128, 1024], F32, tag=f"osb{h}",
                                       name=f"osb{h}") for h in range(2)]
                        for h in range(2):
                            nc.vector.tensor_copy(osb[h][0:65, :],
                                                  poh[h][0:65, :])
                        rd = []
                        for h in range(2):
                            dn = p3.tile([128, 1024], F32, tag=f"dn{h}",
                                         name=f"dn{h}")
                            nc.sync.dma_start(dn[0:1, :], osb[h][64:65, :])
                            nc.vector.reciprocal_approx_fast(
                                dn[0:1, :], dn[0:1, :])
                            rdh = p3.tile([128, 1024], F32, tag=f"rd{h}",
                                          name=f"rd{h}")
                            nc.gpsimd.partition_broadcast(
                                rdh[0:64, :], dn[0:1, :], channels=64)
                            rd.append(rdh)
                        nc.vector.tensor_mul(
                            onorm[p][0:64, nh * 1024:(nh + 1) * 1024],
                            osb[0][0:64, :], rd[0][0:64, :])
                        tmpB = p3.tile([128, 1024], BF16, tag="tmpB")
                        nc.vector.tensor_mul(
                            tmpB[0:64, :], osb[1][0:64, :], rd[1][0:64, :])
                        nc.sync.dma_start(
                            onorm[p][64:128, nh * 1024:(nh + 1) * 1024],
                            tmpB[0:64, :])
                with tc.tile_pool(name="p4", bufs=1) as p4:
                    for nt in range(NT):
                        proj_tiles(p4, ps3, nt)

    nc.compile()
    return nc


def _prep_core_inputs(x, W_qkv, q_gamma, k_gamma, W_proj):
    """Host-side sharding + layout prep. Returns list of 8 in_maps."""
    import ml_dtypes
    bf16 = ml_dtypes.bfloat16
    f32 = np.float32
    blkdiag = np.kron(np.eye(2, dtype=f32), np.ones((64, 64), f32))
    in_maps = []
    for core in range(N_CORES):
        b, g = core // 4, core % 4
        heads = [4 * g + j for j in range(HEADS_PER_CORE)]
        qcols = np.concatenate(
            [(W_qkv[h * HD:(h + 1) * HD, :] * q_gamma[:, None]).T for h in heads],
            axis=1)
        kcols = np.concatenate(
            [(W_qkv[DIM + h * HD:DIM + (h + 1) * HD, :] * k_gamma[:, None]).T
             for h in heads], axis=1)
        wqk = np.ascontiguousarray(
            np.concatenate([qcols, kcols], axis=1)).astype(bf16)
        wv = np.ascontiguousarray(
            np.concatenate(
                [W_qkv[2 * DIM + h * HD:2 * DIM + (h + 1) * HD, :].T
                 for h in heads], axis=1)).astype(bf16)
        wp = np.ascontiguousarray(
            W_proj[:, heads[0] * HD:(heads[-1] + 1) * HD].T).astype(bf16)
        g2q = np.tile(q_gamma, 2).astype(f32)
        g2k = np.tile(k_gamma, 2).astype(f32)
        xTb = np.ascontiguousarray(x[b].T).astype(bf16)
        im = {
            "wqk": wqk, "wv": wv, "wp": wp,
            "smu_q": (blkdiag / 64.0).astype(bf16),
            "ssq_q": (blkdiag / 64.0).astype(bf16),
            "smu_k": (blkdiag / 64.0).astype(bf16),
            "ssq_k": (blkdiag / 64.0).astype(bf16),
            "gq": (1.0 / (g2q * g2q))[:, None].astype(f32),
            "gk": (1.0 / (g2k * g2k))[:, None].astype(f32),
        }
        for i in range(NCH):
            im[f"xT{i}"] = np.ascontiguousarray(xTb[:, i * 512:(i + 1) * 512])
        in_maps.append(im)
    return in_maps


def _numpy_fallback(x, W_qkv, q_gamma, q_beta, k_gamma, k_beta, W_proj, b_proj):
    def ln(t, gamma, beta):
        mu = t.mean(-1, keepdims=True)
        var = ((t - mu) ** 2).mean(-1, keepdims=True)
        return (t - mu) / np.sqrt(var + EPS) * gamma + beta
    Bs, Ns, C = x.shape
    qkv = np.einsum('bnc,oc->bno', x, W_qkv)
    qkv = qkv.reshape(Bs, Ns, 3, HEADS, HD).transpose(2, 0, 3, 1, 4)
    q, k, v = ln(qkv[0], q_gamma, q_beta), ln(qkv[1], k_gamma, k_beta), qkv[2]
    s = np.einsum('bhnd,bhmd->bhnm', q * SCALE, k)
    s = np.exp(s - s.max(-1, keepdims=True))
    p = s / s.sum(-1, keepdims=True)
    o = np.einsum('bhnm,bhmd->bhnd', p, v)
    o = o.transpose(0, 2, 1, 3).reshape(Bs, Ns, C)
    return (np.einsum('bnc,oc->bno', o, W_proj) + b_proj).astype(np.float32)


def kernel(x, W_qkv, q_gamma, q_beta, k_gamma, k_beta, W_proj, b_proj):
    x = np.asarray(x, np.float32)
    W_qkv = np.asarray(W_qkv, np.float32)
    q_gamma = np.asarray(q_gamma, np.float32)
    q_beta = np.asarray(q_beta, np.float32)
    k_gamma = np.asarray(k_gamma, np.float32)
    k_beta = np.asarray(k_beta, np.float32)
    W_proj = np.asarray(W_proj, np.float32)
    b_proj = np.asarray(b_proj, np.float32)

    if np.any(q_beta != 0) or np.any(k_beta != 0):
        # beta terms are not wired into the device kernel (reference always
        # uses beta = 0); fall back to exact host computation
        return _numpy_fallback(x, W_qkv, q_gamma, q_beta, k_gamma, k_beta,
                               W_proj, b_proj)

    from concourse import bass_utils

    if "prog" not in _prog_cache:
        _prog_cache["prog"] = _build_program()
    nc = _prog_cache["prog"]

    in_maps = _prep_core_inputs(x, W_qkv, q_gamma, k_gamma, W_proj)
    res = bass_utils.run_bass_kernel_spmd(nc, in_maps, list(range(N_CORES)))

    out = np.empty((B, N, DIM), np.float32)
    for b in range(B):
        acc = res.results[4 * b + 0]["y"].astype(np.float32).copy()
        for g in range(1, 4):
            acc += res.results[4 * b + g]["y"]
        out[b] = acc + b_proj
    return out
